# revision 13
# baseline (speedup 1.0000x reference)
"""Distributed causal multi-head attention for 8 TRN2 NeuronCores (v3).

Problem: B=4, S=2048, D=1024, H=16 heads of DH=64, fp32 in/out, causal mask.
Sharding: core c -> (batch b = c//2, head-group g = c%2 of 8 heads).

v3 design (v2 @362us -> ~315us measured; ~5us run-to-run variance, first
run after a fresh compile can read ~40us slower):
- heads processed in PAIRS (2p, 2p+1): the two K=64 score matmuls are
  row-split across the PE array (tile_position (0,0)/(64,0)) and run
  CONCURRENTLY -> scores PE time halves (measured 259ns/pair vs 893).
- per-chunk (512-query) granularity: score slot (i, c) writes a 2-bank
  PSUM pair tile (h1 bank | h2 bank); ONE pair-wide exp per slot
  ([128, 2, w] strided ACT call) -> fewer, wider ACT calls.
- ex blocks exb[i] = [128, 1024] f16 (h1|h2), one per key tile, reused
  across chunks (WAR chains via AV reads pipeline exp behind AV).
- emission stream per global chunk: [pending normalize mults] ->
  [AV of previous chunk, both heads] -> [score slots + filler passes].
- filler (Q/K/V projections, O-proj groups) spread across the stream to
  keep the PE dense and HAM-warm; O-proj interleaves into pair 3 with
  pair-3 chunks processed in REVERSE (c3..c0) so the exposed tail is
  only the cheap c0 chunk.
- causal mask by a tiny accumulate matmul (mneg.T @ I adds -1000 above
  the diagonal pre-exp) instead of gpsimd affine_select -- keeps the
  scores->exp->AV chain off the gpsimd FIFO.
- f16 denominator path (stg row 64, transpose-DMA recip, broadcast).
"""

import numpy as np

import concourse.bass as bass
import concourse.mybir as mybir
import concourse.tile as tile
from concourse import bacc

B, S, D, H = 4, 2048, 1024, 16
DH = 64
NG = 2              # head groups (cores per batch)
DG = D // NG        # 512 head dims per core
HL = H // NG        # 8 heads per core
NP = HL // 2        # 4 head pairs per core
PB = 128            # partition block
CH = 512            # free-dim chunk (one fp32 PSUM bank)
NCH = S // CH       # 4 chunks
NKT = S // PB       # 16 key tiles
NDT = D // PB       # 8 contraction tiles for projections
NJT = DG // PB      # 4 head-dim blocks per core (= pairs)
HS = S // 2         # 1024, half of seq
F32 = mybir.dt.float32
F16 = mybir.dt.float16
SCALE = 1.0 / 8.0   # 1/sqrt(DH)


def _emit(nc, xq, xkv, wq, wk, wv, wo, pb, cmask, outT):
    with tile.TileContext(nc) as tc:
        with (
            tc.tile_pool(name="pers", bufs=1) as pers,
            tc.tile_pool(name="xp", bufs=1) as xp,
            tc.tile_pool(name="wp", bufs=1) as wp,
            tc.tile_pool(name="wo", bufs=1) as wop,
            tc.tile_pool(name="qt", bufs=1) as qtp,
            tc.tile_pool(name="kt", bufs=1) as ktp,
            tc.tile_pool(name="vt", bufs=1) as vtp,
            tc.tile_pool(name="at", bufs=1) as attp,
            tc.tile_pool(name="ex", bufs=1) as exp_pool,
            tc.tile_pool(name="rc", bufs=1) as rcp,
            tc.tile_pool(name="ost", bufs=4) as ostp,
            tc.tile_pool(name="ps", bufs=1, space="PSUM") as ps,
        ):
            # ---------------- persistent small tiles ----------------
            pbias_sb = pers.tile([PB, NKT], F32, tag="pbias", name="pbias_sb")
            nc.sync.dma_start(out=pbias_sb[:], in_=pb[:].rearrange("(i p) -> p i", p=PB))

            # ---------------- long-lived activation tiles ----------------
            qt = [qtp.tile([PB, S], F16, tag=f"qt{j}", name=f"qt{j}") for j in range(NJT)]
            kt = [ktp.tile([PB, S], F16, tag=f"kt{j}", name=f"kt{j}") for j in range(NJT)]
            # V with one extra "ones" column per head: (128, 8*65)
            vt = [vtp.tile([PB, HL * (DH + 1)], F16, tag=f"vt{i}", name=f"vt{i}") for i in range(NKT)]
            ones8 = pers.tile([PB, HL], F16, tag="ones8", name="ones8")
            nc.gpsimd.memset(ones8[:], 1.0)
            for i in range(NKT):
                ones_view = vt[i][:].rearrange("p (h c) -> p h c", c=DH + 1)[:, :, DH]
                nc.vector.tensor_copy(ones_view, ones8[:])
            # attention output, transposed: att_sb[p][r, q], r = head-dim row
            # within pair p (heads 2p at rows 0:64, 2p+1 at 64:128)
            att_sb = [attp.tile([PB, S], F16, tag=f"at{j}", name=f"at{j}") for j in range(NJT)]
            # exp blocks: one per key tile, [h1 chunk | h2 chunk]
            exb = [exp_pool.tile([PB, 2 * CH], F16, tag=f"x{i}", name=f"exb{i}") for i in range(NKT)]

            def load_w(dram_w, d, pfx):
                t = wp.tile([PB, DG], F16, tag=f"{pfx}{d}", name=f"{pfx}{d}")
                nc.sync.dma_start(out=t[:], in_=dram_w[d * PB:(d + 1) * PB, :])
                return t

            def load_xq(d, half):
                t = xp.tile([PB, HS], F16, tag=f"b{d}", name=f"xq{d}_{half}")
                nc.sync.dma_start(
                    out=t[:], in_=xq[d * PB:(d + 1) * PB,
                                     half * HS:(half + 1) * HS])
                return t

            def load_kv(d, half):
                t = xp.tile([PB, HS], F16, tag=f"kv{half * NDT + d}", name=f"kv{d}_{half}")
                nc.sync.dma_start(
                    out=t[:], in_=xkv[d * PB:(d + 1) * PB,
                                      half * HS:(half + 1) * HS])
                return t

            # input DMAs (issue order ~= execution order on the sync queue):
            # j0 weight columns first -- only 4.6MB must land before the
            # first score slot instead of 6.1MB.
            def load_w_j0(dram_w, pfx):
                ts = []
                for d in range(NDT):
                    t = wp.tile([PB, DG], F16, tag=f"{pfx}{d}", name=f"{pfx}{d}")
                    nc.sync.dma_start(out=t[:, 0:PB],
                                      in_=dram_w[d * PB:(d + 1) * PB, 0:PB])
                    ts.append(t)
                return ts

            def load_w_rest(ts, dram_w):
                for d in range(NDT):
                    nc.sync.dma_start(out=ts[d][:, PB:DG],
                                      in_=dram_w[d * PB:(d + 1) * PB, PB:DG])

            wq_tiles = load_w_j0(wq, "wq")
            wk_tiles = load_w_j0(wk, "wk")
            # causal-mask matmul operands: mneg (strict-upper -1000, lhsT)
            # and identity; mneg.T @ I adds -1000 at [k, q] for k > q
            xq_cur = [load_xq(d, 0) for d in range(NDT)]   # half 0 resident
            cm_mneg = pers.tile([PB, PB], F16, tag="cmm", name="cm_mneg")
            nc.sync.dma_start(out=cm_mneg[:], in_=cmask[0:PB, :])
            # identity doubled [I | I]: one mask matmul streams 256 columns
            # and drains into BOTH heads' banks via a 2-segment out AP
            cm_id2 = pers.tile([PB, 2 * PB], F16, tag="cmi", name="cm_id2")
            nc.sync.dma_start(out=cm_id2[:, 0:PB], in_=cmask[PB:2 * PB, :])
            nc.sync.dma_start(out=cm_id2[:, PB:2 * PB], in_=cmask[PB:2 * PB, :])
            kv_halves = [[load_kv(d, 0) for d in range(NDT)], None]
            load_w_rest(wq_tiles, wq)
            load_w_rest(wk_tiles, wk)
            wv_tiles = [load_w(wv, d, "wv") for d in range(NDT)]
            kv_halves[1] = [load_kv(d, 1) for d in range(NDT)]

            wol = []

            def load_wol():
                for j in range(NJT):
                    t = wop.tile([PB, D], F16, tag=f"wo{j}", name=f"wo{j}")
                    nc.sync.dma_start(out=t[:], in_=wo[j * PB:(j + 1) * PB, :])
                    wol.append(t)

            # ---------------- PSUM rotation ----------------
            rot = {"n": 0}

            def ptile():
                t = ps.tile([PB, 2 * CH], F32, tag=f"P{rot['n'] % 3}",
                            name=f"pt{rot['n'] % 3}")
                rot["n"] += 1
                return t

            def ab_accs(name):
                return [ps.tile([PB, CH], F32, tag=t, name=name)
                        for t in ("AV", "BV")]

            # ---------------- projection passes (PE filler units) ----------
            def q_pass(j, half):
                accs = ab_accs("qp")
                for d in range(NDT):
                    for ci in range(2):
                        nc.tensor.matmul(
                            accs[ci],
                            wq_tiles[d][:, j * PB:(j + 1) * PB],
                            xq_cur[d][:, ci * CH:(ci + 1) * CH],
                            start=(d == 0), stop=(d == NDT - 1),
                        )
                for ci in range(2):
                    c = half * 2 + ci
                    nc.vector.tensor_copy(qt[j][:, c * CH:(c + 1) * CH], accs[ci])

            def k_pass(j, half):
                accs = ab_accs("kp")
                xh = kv_halves[half]
                for d in range(NDT):
                    for ci in range(2):
                        nc.tensor.matmul(
                            accs[ci],
                            wk_tiles[d][:, j * PB:(j + 1) * PB],
                            xh[d][:, ci * CH:(ci + 1) * CH],
                            start=(d == 0), stop=(d == NDT - 1),
                        )
                for ci in range(2):
                    c = half * 2 + ci
                    nc.vector.tensor_copy(kt[j][:, c * CH:(c + 1) * CH], accs[ci])

            def v_pass(half, spair):
                accs = ab_accs("vp")
                xh = kv_halves[half]
                for d in range(NDT):
                    for s2 in range(2):
                        si = spair * 2 + s2
                        nc.tensor.matmul(
                            accs[s2],
                            xh[d][:, si * PB:(si + 1) * PB],
                            wv_tiles[d][:],
                            start=(d == 0), stop=(d == NDT - 1),
                        )
                for s2 in range(2):
                    i = half * 8 + spair * 2 + s2
                    src = accs[s2].rearrange("p (h c) -> p h c", c=DH)
                    dst = vt[i][:].rearrange("p (h c) -> p h c", c=DH + 1)[:, :, 0:DH]
                    nc.vector.tensor_copy(dst, src)

            def dma_xq_h1():
                # rebind the b-tags to xq half 1 (WAR: after last h0 read)
                for d in range(NDT):
                    xq_cur[d] = load_xq(d, 1)

            # ---------------- attention primitives ----------------
            def emit_slot(p, i, c):
                q_lo = max(c * CH, i * PB)
                off = q_lo - c * CH
                diag = q_lo == i * PB
                pt = ptile()
                nc.tensor.matmul(
                    pt[:, off:CH],
                    kt[p][0:DH, i * PB:(i + 1) * PB],
                    qt[p][0:DH, q_lo:(c + 1) * CH],
                    start=True, stop=not diag, tile_position=(0, 0),
                    skip_group_check=diag,
                )
                nc.tensor.matmul(
                    pt[:, CH + off:2 * CH],
                    kt[p][DH:PB, i * PB:(i + 1) * PB],
                    qt[p][DH:PB, q_lo:(c + 1) * CH],
                    start=True, stop=not diag, tile_position=(64, 0),
                    skip_group_check=diag,
                )
                if diag:
                    # accumulate -1000 above the diagonal of the 128-wide
                    # diagonal block (both heads) -> exp underflows to 0
                    nc.tensor.matmul(
                        pt[:].rearrange("q (h c) -> q h c", c=CH)[:, :, off:off + PB],
                        cm_mneg[:], cm_id2[:],
                        start=False, stop=True, skip_group_check=True,
                    )
                src = pt[:].rearrange("q (h c) -> q h c", c=CH)[:, :, off:CH]
                dst = exb[i][:].rearrange("q (h c) -> q h c", c=CH)[:, :, off:CH]
                nc.scalar.activation(
                    dst, src, mybir.ActivationFunctionType.Exp,
                    bias=pbias_sb[:, i:i + 1], scale=SCALE,
                )

            # per-head staging: rows 0:64 = O numerator, row 64 = denominator
            stg = [rcp.tile([DH + 1, S], F16, tag=f"stg{h}", name=f"stg{h}")
                   for h in range(2)]

            def att_rows(hg):
                return att_sb[hg // 2][(hg % 2) * DH:(hg % 2) * DH + DH, :]

            def emit_av_head(p, c, h):
                acc = ps.tile([PB, CH], F32, tag="AV" if h == 0 else "BV",
                              name=f"av{h}")
                hg = 2 * p + h
                for i in range(4 * c + 4):
                    q_lo = max(c * CH, i * PB)
                    off = q_lo - c * CH
                    nc.tensor.matmul(
                        acc[0:DH + 1, off:CH],
                        vt[i][:, hg * (DH + 1):(hg + 1) * (DH + 1)],
                        exb[i][:, h * CH + off:(h + 1) * CH],
                        start=(i == 0), stop=(i == 4 * c + 3),
                    )
                nc.vector.tensor_copy(stg[h][:, c * CH:(c + 1) * CH],
                                      acc[0:DH + 1, :])

            def head_done(p, h):
                # reciprocal of all 2048 denominators via partition-transpose
                # DMA, then broadcast 1/den across partitions (f16 path)
                dnp = rcp.tile([PB, NKT], F16, tag=f"dnp{h}", name=f"dnp{h}")
                nc.sync.dma_start(out=dnp[:], in_=stg[h][DH:DH + 1, :])
                rcs = rcp.tile([PB, NKT], F16, tag=f"rcs{h}", name=f"rcs{h}")
                with nc.allow_low_precision(reason="softmax recip"):
                    nc.vector.reciprocal(rcs[:], dnp[:])
                rc2 = rcp.tile([1, S], F16, tag=f"rc2{h}", name=f"rc2{h}")
                nc.sync.dma_start(out=rc2[:], in_=rcs[:])
                bc = rcp.tile([DH, S], F16, tag=f"bc{h}", name=f"bc{h}")
                nc.gpsimd.partition_broadcast(bc[:], rc2[0:1, :])
                return bc

            def head_mult(p, h, bc):
                nc.vector.tensor_tensor(att_rows(2 * p + h), stg[h][0:DH, :],
                                        bc[:], mybir.AluOpType.mult)

            bc7 = [None, None]

            def chunk_norm(p, c, h):
                # last pair: normalize per chunk so O-proj can follow.
                # reuses the bc{h} tag — pair-2's head_mult (its last read)
                # precedes the first chunk_norm write in program order.
                if bc7[h] is None:
                    bc7[h] = rcp.tile([DH, S], F16, tag=f"bc{h}", name=f"bc7{h}")
                dnp4 = rcp.tile([PB, NCH], F16, tag=f"dnp4{h}", name=f"dnp4{h}")
                nc.sync.dma_start(out=dnp4[:],
                                  in_=stg[h][DH:DH + 1, c * CH:(c + 1) * CH])
                rcs4 = rcp.tile([PB, NCH], F16, tag=f"rcs4{h}", name=f"rcs4{h}")
                with nc.allow_low_precision(reason="softmax recip"):
                    nc.vector.reciprocal(rcs4[:], dnp4[:])
                rc24 = rcp.tile([1, CH], F16, tag=f"rc24{h}", name=f"rc24{h}")
                nc.sync.dma_start(out=rc24[:], in_=rcs4[:])
                nc.gpsimd.partition_broadcast(bc7[h][:, c * CH:(c + 1) * CH],
                                              rc24[0:1, :])
                nc.vector.tensor_tensor(
                    att_rows(2 * p + h)[:, c * CH:(c + 1) * CH],
                    stg[h][0:DH, c * CH:(c + 1) * CH],
                    bc7[h][:, c * CH:(c + 1) * CH],
                    mybir.AluOpType.mult)

            def _oproj_out(c, m, ost):
                # out-DMAs alternate between the two HW DGE queues: halves
                # the per-queue drain at the tail and the ACT trigger load
                eng = nc.scalar if (c + m) % 2 else nc.sync
                eng.dma_start(
                    out=outT[m * PB:(m + 1) * PB, c * CH:(c + 1) * CH],
                    in_=ost[:])

            def oproj_group(c, m):
                acc = ptile()[:, 0:CH]
                for j in range(NJT):
                    nc.tensor.matmul(
                        acc,
                        wol[j][:, m * PB:(m + 1) * PB],
                        att_sb[j][:, c * CH:(c + 1) * CH],
                        start=(j == 0), stop=(j == NJT - 1),
                    )
                ost = ostp.tile([PB, CH], F16, tag="ost", name="ost")
                nc.vector.tensor_copy(ost[:], acc)
                _oproj_out(c, m, ost)

            # ---------------- schedule ----------------
            # startup: ONLY Q j0 + K j0 before the first score slot (PE is
            # in-order -- anything emitted before slot 0 delays exp).  The
            # remaining Q half-0 passes are early fills; after the last one,
            # the xq tags rebind to half 1 (that DMA waits on the last
            # half-0 read) with the wo loads queued behind it.
            q_pass(0, 0)
            # q10 needs only xq half-0 (already resident when q00 ends);
            # emitting it before k00 fills the PE hole while k00's matmuls
            # wait on the kv half-0 DMA arrivals (k00 still finishes at the
            # same DMA-gated time)
            q_pass(1, 0)
            k_pass(0, 0)

            def q30_and_dma():
                q_pass(3, 0)
                dma_xq_h1()
                load_wol()

            segs = ([(p, c) for p in range(3) for c in range(NCH)]
                    + [(3, c) for c in (3, 2, 1, 0)])

            fills = {
                0: [lambda: v_pass(0, 0), lambda: v_pass(0, 1)],
                1: [lambda: q_pass(2, 0), q30_and_dma,
                    lambda: v_pass(0, 2), lambda: v_pass(0, 3),
                    lambda: k_pass(0, 1), lambda: q_pass(0, 1)],
                2: [lambda: v_pass(1, 0), lambda: v_pass(1, 1),
                    lambda: k_pass(1, 0)],
                3: [lambda: v_pass(1, 2), lambda: v_pass(1, 3),
                    lambda: k_pass(1, 1)],
                4: [lambda: q_pass(1, 1)],
                5: [lambda: k_pass(2, 0)],
                6: [lambda: q_pass(2, 1)],
                7: [lambda: k_pass(2, 1)],
                8: [lambda: q_pass(3, 1)],
                9: [lambda: k_pass(3, 0)],
                10: [lambda: k_pass(3, 1)],
            }

            pending_mults = []

            def finish_chunk(pp, cc):
                """AV + staging for chunk (pp, cc); returns extra fills."""
                extra = []
                emit_av_head(pp, cc, 0)
                emit_av_head(pp, cc, 1)
                if pp == 3:
                    chunk_norm(pp, cc, 0)
                    chunk_norm(pp, cc, 1)
                    extra = [(lambda m=m, cc=cc: oproj_group(cc, m))
                             for m in range(D // PB)]
                elif cc == NCH - 1:
                    for h in range(2):
                        bc = head_done(pp, h)
                        pending_mults.append((pp, h, bc))
                return extra

            for si, (p, c) in enumerate(segs):
                # deferred normalizes (bc chains have had a segment to land)
                while pending_mults:
                    head_mult(*pending_mults.pop(0))
                seg_fills = list(fills.get(si, []))
                if si > 0:
                    seg_fills = finish_chunk(*segs[si - 1]) + seg_fills
                nslots = 4 * c + 4
                nf = len(seg_fills)
                popped = 0
                for i in range(nslots):
                    emit_slot(p, i, c)
                    want = (i + 1) * nf // nslots
                    while popped < want:
                        seg_fills[popped]()
                        popped += 1
            # flush: last chunk (pair 3, chunk 0) + its O-proj groups
            for f in finish_chunk(*segs[-1]):
                f()


def build_module():
    nc = bacc.Bacc()
    xq = nc.declare_dram_parameter("xqT", [D, S], F16, isOutput=False)
    xkv = nc.declare_dram_parameter("xkvT", [D, S], F16, isOutput=False)
    wq = nc.declare_dram_parameter("wqT", [D, DG], F16, isOutput=False)
    wk = nc.declare_dram_parameter("wkT", [D, DG], F16, isOutput=False)
    wv = nc.declare_dram_parameter("wvT", [D, DG], F16, isOutput=False)
    wo = nc.declare_dram_parameter("woT", [DG, D], F16, isOutput=False)
    pb = nc.declare_dram_parameter("pbias", [S], F32, isOutput=False)
    cm = nc.declare_dram_parameter("cmask", [2 * PB, PB], F16, isOutput=False)
    outT = nc.declare_dram_parameter("outT", [D, S], F16, isOutput=True)
    _emit(nc, xq, xkv, wq, wk, wv, wo, pb, cm, outT)
    nc.finalize()
    return nc


_NC = None


def _get_nc():
    global _NC
    if _NC is None:
        _NC = build_module()
    return _NC


def make_in_maps(q_raw, kv_raw, padding_mask, Wq, Wk, Wv, Wo):
    q_raw = np.asarray(q_raw, np.float32)
    kv_raw = np.asarray(kv_raw, np.float32)
    qT = np.ascontiguousarray(q_raw.transpose(0, 2, 1)).astype(np.float16)
    kvT = np.ascontiguousarray(kv_raw.transpose(0, 2, 1)).astype(np.float16)
    pbias = np.where(np.asarray(padding_mask) == 0, -1e9, 0.0).astype(np.float32)
    mneg = np.triu(np.full((PB, PB), -1000.0, np.float16), k=1)
    cmask = np.concatenate([mneg, np.eye(PB, dtype=np.float16)], axis=0)
    Wq, Wk, Wv, Wo = (np.asarray(w, np.float32) for w in (Wq, Wk, Wv, Wo))
    wqT = [np.ascontiguousarray(Wq[g * DG:(g + 1) * DG, :].T).astype(np.float16) for g in range(NG)]
    wkT = [np.ascontiguousarray(Wk[g * DG:(g + 1) * DG, :].T).astype(np.float16) for g in range(NG)]
    wvT = [np.ascontiguousarray(Wv[g * DG:(g + 1) * DG, :].T).astype(np.float16) for g in range(NG)]
    woT = [np.ascontiguousarray(Wo[:, g * DG:(g + 1) * DG].T).astype(np.float16) for g in range(NG)]
    in_maps = []
    for c in range(NG * B):
        b, g = divmod(c, NG)
        in_maps.append({
            "xqT": qT[b], "xkvT": kvT[b],
            "wqT": wqT[g], "wkT": wkT[g], "wvT": wvT[g], "woT": woT[g],
            "pbias": pbias[b], "cmask": cmask,
        })
    return in_maps


def kernel(q_raw, kv_raw, padding_mask, Wq, Wk, Wv, Wo):
    from concourse.bass_utils import run_bass_kernel_spmd

    nc = _get_nc()
    in_maps = make_in_maps(q_raw, kv_raw, padding_mask, Wq, Wk, Wv, Wo)
    res = run_bass_kernel_spmd(nc, in_maps, core_ids=list(range(NG * B)))
    out = np.empty((B, S, D), np.float32)
    for b in range(B):
        out[b] = (res.results[NG * b]["outT"].astype(np.float32)
                  + res.results[NG * b + 1]["outT"].astype(np.float32)).T
    return out
